# revision 24
# baseline (speedup 1.0000x reference)
"""Trainium2 Bass kernel for nn_BCA_17274358465235.

Module: out = x + conv1x1_up( softmax(fx @ fy_up^T) @ fself ) with
fx/fself = 2-layer 1x1-conv projections of x, fy = projection of
bilinearly-upsampled y.  B=4, CX=256, CY=512, CM=64, H=W=64 (N=4096
tokens), HY=WY=32.

Sharding: 8 cores = batch(4) x query-row-half(2).  Each core holds all
4096 keys (fy/fself replicated per batch) and 2048 query rows.  No
collectives.  One program for all cores (SPMD).

Per-core algorithm (layouts chosen so no transposes are needed):
  fself^T[key, c]  via second projection layer emitted transposed
  sim^T[key, row] = fy_f[:, keys]^T @ fx[:, rows]   (fp32r matmuls,
      two key-chunks packed into PE row-groups 0-1 / 2-3)
  exp: split between ACT (exact, bf16 out) and DVE (Schraudolph:
      int16(A*sim+B) bit-cast as bf16; end-to-end rel err stays ~1.2e-2)
  fout^T[c, row] += fself^T_chunk^T @ exp_chunk   (PSUM accumulation,
      ones-column in fself^T produces the softmax denominator Z free)
  out = x + W_up @ (fout^T * (1/Z)) + b_up   (b_up via ones-row in W_up)

Performance changes over the first working kernel:
  - exp split ACT/DVE: the ACT engine alone paced the main loop at
    ~1.22us/tile; DVE takes a share via the Schraudolph step.
  - half-0's 1/Z scaling + up-projection + residual + output DMA are
    hooked into half-1's attention loop; only half 1's tail is serial.
  - fy upsample fused with scalar_tensor_tensor (drops the 0.75-scaled
    copy and half the fyc work); W-pass bands 1-3 + the fx path are
    ordered so the first sims don't wait on the whole fy chain (deps
    lower to monotonic per-engine op-count waits).
  - no PE warm-up/keepalive matmuls: the PE is power duty-cycle capped
    (~70%; ~20.5us half-rate windows under sustained load), so dummy
    matmuls cost real budget.
"""
import sys

for _p in ("/opt/pypackages", "/opt/trn_rl_repo"):
    if _p not in sys.path:
        sys.path.insert(0, _p)

import numpy as np

import concourse.bacc as bacc
import concourse.mybir as mybir
import concourse.tile as tile
from concourse.bass_utils import run_bass_kernel_spmd

F32 = mybir.dt.float32
F32R = mybir.dt.float32r
F16 = mybir.dt.float16
BF16 = mybir.dt.bfloat16
I16 = mybir.dt.int16
EXP = mybir.ActivationFunctionType.Exp
COPY = mybir.ActivationFunctionType.Copy
IDENT = mybir.ActivationFunctionType.Identity
MUL = mybir.AluOpType.mult
ADD = mybir.AluOpType.add

B, CX, CY, CM = 4, 256, 512, 64
H = W = 64
HY = WY = 32
N = H * W              # 4096 tokens
NH = N // 2            # 2048 query rows per core
NYC = HY * WY          # 1024 coarse tokens
KC = N // 128          # 32 key chunks

# Schraudolph exp in bf16 bit-domain: bf16_bits(e^x) ~ int16(A16*x + B16)
# (the DVE f32->int16 output conversion rounds to nearest).
A16 = float((1 << 7) / np.log(2.0))
B16 = 16250.12

# Which iteration indices (0..31) of each half-loop run exp on DVE
# (Schraudolph); the rest run exact exp on ACT.  DVE also carries the
# fselfT copies + deferred fy W-pass bands (half 0) and the pre-tail /
# residual work (half 1), so its share starts late in each half.
DVE_H0 = frozenset(j for j in range(KC) if j % 2 == 1 and 5 <= j < 29)
DVE_H1 = frozenset(j for j in range(KC) if j % 2 == 1 and 5 <= j < 28)

_CACHE = {}


def _build():
    nc = bacc.Bacc("TRN2", target_bir_lowering=False, debug=False,
                   enable_asserts=False)

    # ---- DRAM I/O (per-core layouts pre-arranged on host) ----
    # xs: [128, 8 * 1024] block-major: block g = [ch0-127 | ch128-255] of
    #     pixel columns g*512..(g+1)*512  (for fself over the full image)
    xs = nc.dram_tensor("xs", [128, 8192], F16, kind="ExternalInput").ap()
    # xl: [128, 2 * 2048] ch-chunk-major: this core's 2048 query pixels
    xl = nc.dram_tensor("xl", [128, 4096], F16, kind="ExternalInput").ap()
    # yb: [128, 4 * 1024] ch-chunk-major
    yb = nc.dram_tensor("yb", [128, 4096], F16, kind="ExternalInput").ap()
    wpack = nc.dram_tensor("wpack", [128, 1093], F16, kind="ExternalInput").ap()
    bias32 = nc.dram_tensor("bias32", [64, 3], F32, kind="ExternalInput").ap()
    ones = nc.dram_tensor("ones", [1, 4096], F16, kind="ExternalInput").ap()
    # out: [128, 2 * 2048] ch-chunk-major
    out = nc.dram_tensor("out", [128, 4096], F32, kind="ExternalOutput").ap()

    with tile.TileContext(nc) as tc:
        with tc.tile_pool(name="sbW", bufs=1) as sbW, \
             tc.tile_pool(name="sbM", bufs=1) as sbM:
            # ---- long-lived SBUF ----
            t_xs = sbM.tile([128, 8192], F16)      # full x for fself stream
            t_xl = sbM.tile([128, 4096], F16)      # fx input + residual
            fy2 = sbM.tile([128, 4096], F16)       # upsampled fy, duplicated
            fx2 = sbM.tile([128, 2048], F16)       # fx, duplicated
            fselfT = sbM.tile([128, 65 * KC], BF16)
            h1s_aug = sbM.tile([65, 4096], F16)    # W_self1 @ x with ones row
            scaled = sbM.tile([65, 2048], F16)     # [Z/Z; fout/Z] per row
            t_bias = sbM.tile([64, 3], F32)        # bx2 | 0.75*by2 | 0.25*by2

            # ---- weights (single packed blob) ----
            t_wpack = sbW.tile([128, 1093], F16)
            t_ws1t = t_wpack[:, 0:128]
            t_ws2a = t_wpack[0:65, 128:194]
            t_wx1t = t_wpack[:, 194:322]
            t_wx2t = t_wpack[0:64, 322:386]
            t_wy1t = t_wpack[:, 387:643]
            t_wy2t = t_wpack[0:64, 643:707]
            t_wupt = t_wpack[0:65, 709:965]
            t_bx2 = t_bias[:, 0:1]
            t_by2a = t_bias[:, 1:3]

            # ================= phase 1: projections =================
            with tc.tile_pool(name="sbP", bufs=1) as sbP, \
                 tc.tile_pool(name="psP1", bufs=1, space="PSUM") as psP1:
                # input DMAs, critical-path first
                # DMA priority: weights needed by the early
                # projections, then xs blocks 0-1 (the fself preamble
                # matmuls run first and warm the PE HAM clock gate with
                # real work), then yb for the fy chain, then the rest.
                nc.sync.dma_start(t_wpack[:, 0:709], wpack[:, 0:709])
                nc.sync.dma_start(t_bias[:], bias32[:])
                nc.sync.dma_start(h1s_aug[64:65, :], ones[:, 0:4096])
                for g in range(2):
                    nc.sync.dma_start(t_xs[:, g * 1024:(g + 1) * 1024],
                                      xs[:, g * 1024:(g + 1) * 1024])
                t_yb = sbP.tile([128, 4096], F16)
                for c in range(4):
                    nc.sync.dma_start(t_yb[:, c * 512:(c + 1) * 512],
                                      yb[:, c * 512:(c + 1) * 512])
                nc.sync.dma_start(t_yb[:, 2048:3072], yb[:, 2048:3072])
                nc.sync.dma_start(t_yb[:, 3072:4096], yb[:, 3072:4096])
                for c in range(4):
                    nc.sync.dma_start(t_xl[:, c * 1024:(c + 1) * 1024],
                                      xl[:, c * 1024:(c + 1) * 1024])
                nc.sync.dma_start(t_wpack[:, 709:1093], wpack[:, 709:1093])
                for g in range(2, 8):
                    nc.sync.dma_start(t_xs[:, g * 1024:(g + 1) * 1024],
                                      xs[:, g * 1024:(g + 1) * 1024])

                # NOTE: no PE warm-up matmuls.  The PE is duty-cycle
                # limited (~70% sustained util; the firmware inserts
                # ~20.5us half-rate windows under load), so dummy
                # matmuls burn real budget and slow the whole kernel.

                # fself preamble first: these matmuls depend only on
                # the first DMAs and warm the PE HAM clock gate with
                # real work, so the projections run at full clock.
                for idx in range(2):
                    p = psP1.tile([64, 512], F32, tag="blk", bufs=4,
                                  name=f"pp_h1s_{idx}")
                    for a in range(2):
                        nc.tensor.matmul(p[:], t_ws1t[:, a * 64:(a + 1) * 64],
                                         t_xs[:, idx * 1024 + a * 512:
                                              idx * 1024 + a * 512 + 512],
                                         start=(a == 0), stop=(a == 1))
                    nc.vector.tensor_copy(
                        h1s_aug[0:64, idx * 512:idx * 512 + 512], p[:])
                for j in range(2):
                    p2 = psP1.tile([128, 66], F32, tag="blk", bufs=4,
                                   name=f"pp_fs_{j}")
                    nc.tensor.matmul(p2[:], h1s_aug[:, j * 128:(j + 1) * 128],
                                     t_ws2a, start=True, stop=True)
                    nc.vector.tensor_copy(fselfT[:, j * 65:(j + 1) * 65],
                                          p2[:, 0:65])

                # warm the ACT exp table early
                t_dum = sbP.tile([1, 32], F32)
                nc.vector.memset(t_dum[:], 0.0)
                t_dum2 = sbP.tile([1, 32], F32)
                nc.scalar.activation(t_dum2[:], t_dum[:], EXP)

                # ---- fy path: h1y = Wy1 @ y ; g = Wy2 @ h1y + by2 (raw)
                # and t2 = 0.25*g; banded upsample via fused
                # scalar_tensor_tensor (out = 0.75*a + 0.25-scaled b) ----
                h1y_s = sbP.tile([64, 1024], F16)
                fyc75 = sbP.tile([64, 1024], F32)
                fyc25 = sbP.tile([64, 1024], F32)
                for blk in range(2):
                    p = psP1.tile([64, 512], F32, tag="blk", bufs=4,
                                  name=f"p_h1y_{blk}")
                    for a in range(4):
                        nc.tensor.matmul(
                            p[:], t_wy1t[:, a * 64:(a + 1) * 64],
                            t_yb[:, blk * 2048 + a * 512:blk * 2048 + a * 512 + 512],
                            start=(a == 0), stop=(a == 3))
                    nc.scalar.activation(h1y_s[:, blk * 512:blk * 512 + 512],
                                         p[:], COPY)
                for blk in range(2):
                    p = psP1.tile([64, 512], F32, tag="blk", bufs=4,
                                  name=f"p_fyc_{blk}")
                    nc.tensor.matmul(p[:], t_wy2t,
                                     h1y_s[:, blk * 512:blk * 512 + 512],
                                     start=True, stop=True)
                    bs = slice(blk * 512, blk * 512 + 512)
                    nc.scalar.activation(fyc75[:, bs], p[:], IDENT,
                                         bias=t_by2a[:, 0:1], scale=0.75)
                    nc.scalar.activation(fyc25[:, bs], p[:], IDENT,
                                         bias=t_by2a[:, 1:2], scale=0.25)

                # H pass, 2 bands: [64, (32,32)] -> [64, (64,32)]
                fyH = sbM.tile([64, 2048], F32)
                t1v = fyc75[:].rearrange("p (h w) -> p h w", h=32)
                t2v = fyc25[:].rearrange("p (h w) -> p h w", h=32)
                fe = fyH[:].rearrange("p (h two w) -> p h two w", h=32, two=2)
                GADD = nc.gpsimd.tensor_add
                GADD(fe[:, 0, 0, :], t1v[:, 0, :], t2v[:, 0, :])
                GADD(fe[:, 1:16, 0, :], t1v[:, 1:16, :], t2v[:, 0:15, :])
                GADD(fe[:, 0:15, 1, :], t1v[:, 0:15, :], t2v[:, 1:16, :])
                GADD(fe[:, 16:32, 0, :], t1v[:, 16:32, :], t2v[:, 15:31, :])
                GADD(fe[:, 15:31, 1, :], t1v[:, 15:31, :], t2v[:, 16:32, :])
                GADD(fe[:, 31, 1, :], t1v[:, 31, :], t2v[:, 31, :])

                # 0.25-scaled fyH copy, 2 bands: rows 0..30 / 31..63
                u2 = sbM.tile([64, 2048], F32)
                u2v = u2[:].rearrange("p (h w) -> p h w", h=64)
                fyHv = fyH[:].rearrange("p (h w) -> p h w", h=64)
                nc.scalar.activation(u2[:, 0:31 * 32], fyH[:, 0:31 * 32],
                                     COPY, scale=0.25)
                nc.scalar.activation(u2[:, 31 * 32:2048], fyH[:, 31 * 32:2048],
                                     COPY, scale=0.25)

                # W pass + row-group duplication in 4 h-bands.  Band 0
                # (key chunks 0-7) is emitted at the end of phase 1; the
                # rest stream through half-0's hook so the first sims
                # are not blocked behind the whole fy chain on DVE.
                fw = fy2[0:64, :].rearrange("p (h w two) -> p h w two",
                                            h=64, two=2)
                _BANDS = ((slice(0, 16), 0, 1024),
                          (slice(16, 31), 1024, 1984),
                          (slice(31, 48), 1984, 3072),
                          (slice(48, 64), 3072, 4096))

                STT = nc.vector.scalar_tensor_tensor

                def wpass_band(b):
                    # STT stays on DVE: fy2 is f32r and its writers must
                    # produce rounded f32r (BIR verifier), which GpSimd
                    # only does via a slow software path.  The row-group
                    # duplication goes to ACT (also f32r-capable).
                    hs, c0, c1 = _BANDS[b]
                    nc.vector.tensor_copy(fw[:, hs, 0, 0], fyHv[:, hs, 0])
                    STT(fw[:, hs, 1:32, 0], fyHv[:, hs, 1:32], 0.75,
                        u2v[:, hs, 0:31], MUL, ADD)
                    STT(fw[:, hs, 0:31, 1], fyHv[:, hs, 0:31], 0.75,
                        u2v[:, hs, 1:32], MUL, ADD)
                    nc.vector.tensor_copy(fw[:, hs, 31, 1], fyHv[:, hs, 31])
                    nc.scalar.activation(fy2[64:128, c0:c1],
                                         fy2[0:64, c0:c1], COPY)

                # band 0 immediately -- before the fx path -- so the
                # first sims' ACT/DVE op-count waits don't extend past
                # the fx chain (which depends on later DMAs).
                wpass_band(0)

                # ---- fx path: h1x = Wx1 @ xl ; fx = Wx2 @ h1x + bx2 ----
                # Emitted BEFORE the W pass: the tile framework lowers
                # cross-engine deps as monotonic op-count waits, so the
                # first sims wait for the latest DVE op they depend on.
                # fx2 (incl. duplication) must precede the W-pass bands
                # in the DVE queue or sims stall on the whole fy chain.
                h1x_s = sbP.tile([64, 2048], F16)
                for blk in range(4):
                    p = psP1.tile([64, 512], F32, tag="blk", bufs=4,
                                  name=f"p_h1x_{blk}")
                    for a in range(2):
                        nc.tensor.matmul(
                            p[:], t_wx1t[:, a * 64:(a + 1) * 64],
                            t_xl[:, blk * 1024 + a * 512:blk * 1024 + a * 512 + 512],
                            start=(a == 0), stop=(a == 1))
                    nc.scalar.activation(h1x_s[:, blk * 512:blk * 512 + 512],
                                         p[:], COPY)
                for blk in range(4):
                    p = psP1.tile([64, 512], F32, tag="blk", bufs=4,
                                  name=f"p_fx_{blk}")
                    nc.tensor.matmul(p[:], t_wx2t,
                                     h1x_s[:, blk * 512:blk * 512 + 512],
                                     start=True, stop=True)
                    nc.vector.tensor_scalar_add(fx2[0:64, blk * 512:blk * 512 + 512],
                                                p[:], t_bx2)
                nc.scalar.activation(fx2[64:128, 0:1024],
                                      fx2[0:64, 0:1024], COPY)
                nc.scalar.activation(fx2[64:128, 1024:2048],
                                      fx2[0:64, 1024:2048], COPY)

                # more of the fself stream while the fy chain finishes
                # on ACT/DVE/GpSimd (PE is only ~60% busy here); blocks
                # 6-7 stay in the half-0 hook since their xs DMAs land
                # last.
                for idx in range(2, 6):
                    p = psP1.tile([64, 512], F32, tag="blk", bufs=4,
                                  name=f"pp_h1s_{idx}")
                    for a in range(2):
                        nc.tensor.matmul(p[:], t_ws1t[:, a * 64:(a + 1) * 64],
                                         t_xs[:, idx * 1024 + a * 512:
                                              idx * 1024 + a * 512 + 512],
                                         start=(a == 0), stop=(a == 1))
                    nc.vector.tensor_copy(
                        h1s_aug[0:64, idx * 512:idx * 512 + 512], p[:])
                    for j in (2 * idx - 2, 2 * idx - 1):
                        p2 = psP1.tile([128, 66], F32, tag="blk", bufs=4,
                                       name=f"pp_fs_{j}")
                        nc.tensor.matmul(p2[:],
                                         h1s_aug[:, j * 128:(j + 1) * 128],
                                         t_ws2a, start=True, stop=True)
                        nc.vector.tensor_copy(fselfT[:, j * 65:(j + 1) * 65],
                                              p2[:, 0:65])

            # ================= phase 2: attention =================
            fout_accs = {}

            def sim_unit(pool, j, h):
                ps = pool.tile([128, 1024], F32, tag="sim", bufs=2,
                               name=f"sim_{j}_{h}")
                nc.tensor.matmul(
                    ps[:, 0:512], fy2[0:64, j * 128:(j + 1) * 128],
                    fx2[0:64, h * 1024:h * 1024 + 512],
                    start=True, stop=True)
                nc.tensor.matmul(
                    ps[:, 512:1024], fy2[64:128, j * 128:(j + 1) * 128],
                    fx2[64:128, h * 1024 + 512:h * 1024 + 1024],
                    start=True, stop=True)
                return ps

            def exp_unit(st, j, h, on_dve):
                et = sbM.tile([128, 1024], BF16, tag="et", bufs=4,
                              name=f"et_{j}_{h}")
                if on_dve:
                    nc.vector.tensor_scalar(et[:].bitcast(I16), st[:],
                                            A16, B16, MUL, ADD)
                else:
                    nc.scalar.activation(et[:], st[:], EXP)
                return et

            def pv_unit(fout_acc, et, j):
                w = fselfT[:, j * 65:(j + 1) * 65]
                nc.tensor.matmul(fout_acc[:, 0:512], w, et[:, 0:512],
                                 start=(j == 0), stop=(j == KC - 1))
                nc.tensor.matmul(fout_acc[:, 512:1024], w, et[:, 512:1024],
                                 start=(j == 0), stop=(j == KC - 1))

            def half_loop(h, psB, hook, dve_set):
                fout_acc = fout_accs[h]
                sims = {}
                sims[0] = sim_unit(psB, 0, h)
                sims[1] = sim_unit(psB, 1, h)
                sims[2] = sim_unit(psB, 2, h)
                for j in range(KC):
                    if hook is not None:
                        hook(j)
                    et = exp_unit(sims.pop(j), j, h, j in dve_set)
                    pv_unit(fout_acc, et, j)
                    if j + 3 < KC:
                        sims[j + 3] = sim_unit(psB, j + 3, h)

            def pre_tail(h):
                # 1/Z scaling of fout into `scaled`; recip+broadcast for
                # both column groups first so the gpsimd broadcasts
                # overlap, then the two DVE muls.
                fout_acc = fout_accs[h]
                invzbs = []
                for s in range(2):
                    cs = slice(s * 512, (s + 1) * 512)
                    invz = sbM.tile([1, 512], F32, tag="zrow", bufs=2,
                                    name=f"invz_{h}_{s}")
                    nc.vector.reciprocal_approx_fast(invz[:], fout_acc[0:1, cs])
                    invzb = sbM.tile([128, 512], F32, tag="izb", bufs=2,
                                     name=f"invzb_{h}_{s}")
                    nc.gpsimd.partition_broadcast(invzb[:], invz[:])
                    invzbs.append(invzb)
                for s in range(2):
                    cs = slice(s * 512, (s + 1) * 512)
                    nc.vector.tensor_mul(
                        scaled[:, h * 1024 + s * 512:h * 1024 + (s + 1) * 512],
                        fout_acc[:, cs], invzbs[s][0:65, :])

            def up_quarter(psC, q):
                # up-projection + residual + output DMA for 512 query px
                for a in range(2):
                    p = psC.tile([128, 512], F32, tag="up", bufs=2,
                                 name=f"p_up_{q}_{a}")
                    nc.tensor.matmul(p[:], t_wupt[:, a * 128:(a + 1) * 128],
                                     scaled[:, q * 512:(q + 1) * 512],
                                     start=True, stop=True)
                    out_s = sbM.tile([128, 512], F32, tag="tail", bufs=4,
                                     name=f"out_s_{q}_{a}")
                    xv = t_xl[:, q * 1024 + a * 512:
                              q * 1024 + a * 512 + 512]
                    nc.vector.tensor_add(out_s[:], p[:], xv)
                    nc.sync.dma_start(
                        out[:, a * 2048 + q * 512:a * 2048 + (q + 1) * 512],
                        out_s[:])

            with tc.tile_pool(name="psA0", bufs=1, space="PSUM") as psA0:
                fout_accs[0] = psA0.tile([65, 1024], F32, name="fout0")
                with tc.tile_pool(name="psFS", bufs=1, space="PSUM") as psFS:

                    def fs_mms(j):
                        p = psFS.tile([128, 66], F32, tag="fs", bufs=2,
                                      name=f"p_fs_{j}")
                        nc.tensor.matmul(p[:],
                                         h1s_aug[:, j * 128:(j + 1) * 128],
                                         t_ws2a, start=True, stop=True)
                        nc.vector.tensor_copy(fselfT[:, j * 65:(j + 1) * 65],
                                              p[:, 0:65])

                    def h1s_mms(idx):
                        p = psFS.tile([64, 512], F32, tag="fs", bufs=2,
                                      name=f"p_h1s_{idx}")
                        for a in range(2):
                            nc.tensor.matmul(p[:],
                                             t_ws1t[:, a * 64:(a + 1) * 64],
                                             t_xs[:, idx * 1024 + a * 512:
                                                  idx * 1024 + a * 512 + 512],
                                             start=(a == 0), stop=(a == 1))
                        nc.vector.tensor_copy(
                            h1s_aug[0:64, idx * 512:idx * 512 + 512], p[:])

                    def hook0(j):
                        if j == 0:
                            wpass_band(1)
                        elif j == 4:
                            wpass_band(2)
                        elif j == 10:
                            wpass_band(3)
                        if j < 2:
                            h1s_mms(6 + j)
                        if 10 + 2 * j < KC:
                            fs_mms(10 + 2 * j)
                            fs_mms(11 + 2 * j)

                    with tc.tile_pool(name="psB0", bufs=1,
                                      space="PSUM") as psB0:
                        half_loop(0, psB0, hook0, DVE_H0)

                pre_tail(0)

            with tc.tile_pool(name="psA1", bufs=1, space="PSUM") as psA1:
                fout_accs[1] = psA1.tile([65, 1024], F32, name="fout1")
                with tc.tile_pool(name="psC", bufs=1, space="PSUM") as psC:

                    def hook1(j):
                        # half-0 tail interleaved into half-1's loop
                        if j == 6:
                            up_quarter(psC, 0)
                        elif j == 12:
                            up_quarter(psC, 1)

                    with tc.tile_pool(name="psB1", bufs=1,
                                      space="PSUM") as psB1:
                        half_loop(1, psB1, hook1, DVE_H1)
                    pre_tail(1)
                    up_quarter(psC, 2)
                    up_quarter(psC, 3)

    nc.compile()
    return nc


def _prep_maps(x, y, W_self1, b_self1, W_self2, b_self2, W_x1, b_x1, W_x2,
               b_x2, W_y1, b_y1, W_y2, b_y2, W_up, b_up):
    f64 = np.float64

    def fold(W2, b1, b2):
        return (W2.astype(f64) @ b1.astype(f64) + b2.astype(f64)).astype(np.float32)

    ws2a = np.zeros((65, 66), np.float16)
    ws2a[64, 0] = 1.0
    ws2a[0:64, 1:65] = W_self2.T.astype(np.float16)
    ws2a[64, 1:65] = fold(W_self2, b_self1, b_self2).astype(np.float16)
    bx2 = fold(W_x2, b_x1, b_x2).reshape(64, 1)
    _by2 = fold(W_y2, b_y1, b_y2).astype(np.float64)
    bias32 = np.ascontiguousarray(
        np.concatenate([bx2, 0.75 * _by2.reshape(64, 1),
                        0.25 * _by2.reshape(64, 1)], axis=1).astype(np.float32))

    ws1t = np.ascontiguousarray(
        W_self1.T.reshape(2, 128, 64).transpose(1, 0, 2).reshape(128, 128))
    wx1t = np.ascontiguousarray(
        W_x1.T.reshape(2, 128, 64).transpose(1, 0, 2).reshape(128, 128))
    wy1t = np.ascontiguousarray(
        W_y1.T.reshape(4, 128, 64).transpose(1, 0, 2).reshape(128, 256))
    wx2t = np.ascontiguousarray(W_x2.T)
    wy2t = np.ascontiguousarray(W_y2.T)
    wupt = np.ascontiguousarray(
        np.concatenate([b_up.reshape(1, 256), W_up.T], axis=0))
    wp = np.zeros((128, 1093), np.float16)
    wp[:, 0:128] = ws1t.astype(np.float16)
    wp[0:65, 128:194] = ws2a
    wp[:, 194:322] = wx1t.astype(np.float16)
    wp[0:64, 322:386] = wx2t.astype(np.float16)
    wp[:, 387:643] = wy1t.astype(np.float16)
    wp[0:64, 643:707] = wy2t.astype(np.float16)
    wp[0:65, 709:965] = wupt.astype(np.float16)

    _ONES = np.ones((1, 4096), np.float16)
    maps = []
    for b in range(B):
        xf = x[b].reshape(CX, N).astype(np.float16)             # [256, 4096]
        xs_h = np.ascontiguousarray(
            xf.reshape(2, 128, 8, 512).transpose(1, 2, 0, 3).reshape(128, 8192))
        yf = y[b].reshape(CY, NYC).astype(np.float16)
        yb_h = np.ascontiguousarray(
            yf.reshape(4, 128, 2, 512).transpose(1, 2, 0, 3).reshape(128, 4096))
        for half in range(2):
            xh = xf[:, half * NH:(half + 1) * NH]               # [256, 2048]
            xl_h = np.ascontiguousarray(
                xh.reshape(2, 128, 4, 512).transpose(1, 2, 0, 3).reshape(128, 4096))
            maps.append({
                "xs": xs_h, "xl": xl_h, "yb": yb_h,
                "wpack": wp, "bias32": bias32, "ones": _ONES,
            })
    return maps


def _run(inputs, trace=False, trace_kwargs=None):
    if "nc" not in _CACHE:
        _CACHE["nc"] = _build()
    nc = _CACHE["nc"]
    maps = _prep_maps(**inputs)
    res = run_bass_kernel_spmd(nc, maps, list(range(8)), trace=trace,
                               **(trace_kwargs or {}))
    outs = np.empty((B, CX, H, W), np.float32)
    for b in range(B):
        for half in range(2):
            o = res.results[2 * b + half]["out"]                # [128, 4096]
            oh = o.reshape(128, 2, NH).transpose(1, 0, 2).reshape(CX, NH)
            outs[b, :, :, :].reshape(CX, N)[:, half * NH:(half + 1) * NH] = oh
    return outs, res


def kernel(**inputs):
    outs, _ = _run(inputs, trace=False)
    return outs


# revision 27
# speedup vs baseline: 1.1817x; 1.1817x over previous
"""Trainium2 Bass kernel for nn_BCA_17274358465235.

Module: out = x + conv1x1_up( softmax(fx @ fy_up^T) @ fself ) with
fx/fself = 2-layer 1x1-conv projections of x, fy = projection of
bilinearly-upsampled y.  B=4, CX=256, CY=512, CM=64, H=W=64 (N=4096
tokens), HY=WY=32.

Sharding: 8 cores = batch(4) x query-row-half(2).  Each core holds all
4096 keys (fy/fself replicated per batch) and 2048 query rows.  No
collectives.  One program for all cores (SPMD).

Per-core algorithm (layouts chosen so no transposes are needed):
  fself^T[key, c]  via second projection layer emitted transposed
  sim^T[key, row] = fy_f[:, keys]^T @ fx[:, rows]   (fp32r matmuls,
      two key-chunks packed into PE row-groups 0-1 / 2-3)
  exp: split between ACT (exact, bf16 out) and DVE (Schraudolph:
      int16(A*sim+B) bit-cast as bf16; end-to-end rel err stays ~1.2e-2)
  fout^T[c, row] += fself^T_chunk^T @ exp_chunk   (PSUM accumulation,
      ones-column in fself^T produces the softmax denominator Z free)
  out = x + W_up @ (fout^T * (1/Z)) + b_up   (b_up via ones-row in W_up)

Performance changes over the first working kernel:
  - exp split ACT/DVE: the ACT engine alone paced the main loop at
    ~1.22us/tile; DVE takes a share via the Schraudolph step.
  - half-0's 1/Z scaling + up-projection + residual + output DMA are
    hooked into half-1's attention loop; only half 1's tail is serial.
  - fy upsample fused with scalar_tensor_tensor (drops the 0.75-scaled
    copy and half the fyc work); W-pass bands 1-3 + the fx path are
    ordered so the first sims don't wait on the whole fy chain (deps
    lower to monotonic per-engine op-count waits).
  - no PE warm-up/keepalive matmuls: the PE is power duty-cycle capped
    (~70%; ~20.5us half-rate windows under sustained load), so dummy
    matmuls cost real budget.
"""
import sys

for _p in ("/opt/pypackages", "/opt/trn_rl_repo"):
    if _p not in sys.path:
        sys.path.insert(0, _p)

import numpy as np

import concourse.bacc as bacc
import concourse.mybir as mybir
import concourse.tile as tile
from concourse.bass_utils import run_bass_kernel_spmd

F32 = mybir.dt.float32
F32R = mybir.dt.float32r
F16 = mybir.dt.float16
BF16 = mybir.dt.bfloat16
I16 = mybir.dt.int16
EXP = mybir.ActivationFunctionType.Exp
COPY = mybir.ActivationFunctionType.Copy
IDENT = mybir.ActivationFunctionType.Identity
MUL = mybir.AluOpType.mult
ADD = mybir.AluOpType.add

B, CX, CY, CM = 4, 256, 512, 64
H = W = 64
HY = WY = 32
N = H * W              # 4096 tokens
NH = N // 2            # 2048 query rows per core
NYC = HY * WY          # 1024 coarse tokens
KC = N // 128          # 32 key chunks

# Schraudolph exp in bf16 bit-domain: bf16_bits(e^x) ~ int16(A16*x + B16)
# (the DVE f32->int16 output conversion rounds to nearest).
A16 = float((1 << 7) / np.log(2.0))
B16 = 16250.12

# Which iteration indices (0..31) of each half-loop run exp on DVE
# (Schraudolph); the rest run exact exp on ACT.  DVE also carries the
# fselfT copies + deferred fy W-pass bands (half 0) and the pre-tail /
# residual work (half 1), so its share starts late in each half.
DVE_H0 = frozenset(j for j in range(KC) if j % 2 == 1 and 5 <= j < 29)
DVE_H1 = frozenset(j for j in range(KC) if j % 2 == 1 and 5 <= j < 28)

_CACHE = {}


def _build():
    nc = bacc.Bacc("TRN2", target_bir_lowering=False, debug=False,
                   enable_asserts=False)

    # ---- DRAM I/O (per-core layouts pre-arranged on host) ----
    # xs: [128, 8 * 1024] block-major: block g = [ch0-127 | ch128-255] of
    #     pixel columns g*512..(g+1)*512  (for fself over the full image)
    xs = nc.dram_tensor("xs", [128, 8192], F16, kind="ExternalInput").ap()
    # xl: [128, 2 * 2048] ch-chunk-major: this core's 2048 query pixels
    xl = nc.dram_tensor("xl", [128, 4096], F16, kind="ExternalInput").ap()
    # yb: [128, 4 * 1024] ch-chunk-major
    yb = nc.dram_tensor("yb", [128, 4096], F16, kind="ExternalInput").ap()
    wpack = nc.dram_tensor("wpack", [128, 1093], F16, kind="ExternalInput").ap()
    bias32 = nc.dram_tensor("bias32", [64, 3], F32, kind="ExternalInput").ap()
    ones = nc.dram_tensor("ones", [1, 4096], F16, kind="ExternalInput").ap()
    # out: [128, 2 * 2048] ch-chunk-major
    out = nc.dram_tensor("out", [128, 4096], F32, kind="ExternalOutput").ap()

    with tile.TileContext(nc) as tc:
        with tc.tile_pool(name="sbW", bufs=1) as sbW, \
             tc.tile_pool(name="sbM", bufs=1) as sbM:
            # ---- long-lived SBUF ----
            t_xs = sbM.tile([128, 8192], F16)      # full x for fself stream
            t_xl = sbM.tile([128, 4096], F16)      # fx input + residual
            fy2 = sbM.tile([128, 4096], F16)       # upsampled fy, duplicated
            fx2 = sbM.tile([128, 2048], F16)       # fx, duplicated
            fselfT = sbM.tile([128, 65 * KC], BF16)
            h1s_aug = sbM.tile([65, 4096], F16)    # W_self1 @ x with ones row
            scaled = sbM.tile([65, 2048], F16)     # [Z/Z; fout/Z] per row
            t_bias = sbM.tile([64, 3], F32)        # bx2 | 0.75*by2 | 0.25*by2

            # ---- weights (single packed blob) ----
            t_wpack = sbW.tile([128, 1093], F16)
            t_ws1t = t_wpack[:, 0:128]
            t_ws2a = t_wpack[0:65, 128:194]
            t_wx1t = t_wpack[:, 194:322]
            t_wx2t = t_wpack[0:64, 322:386]
            t_wy1t = t_wpack[:, 387:643]
            t_wy2t = t_wpack[0:64, 643:707]
            t_wupt = t_wpack[0:65, 709:965]
            t_bx2 = t_bias[:, 0:1]
            t_by2a = t_bias[:, 1:3]

            # ================= phase 1: projections =================
            with tc.tile_pool(name="sbP", bufs=1) as sbP, \
                 tc.tile_pool(name="psP1", bufs=1, space="PSUM") as psP1:
                # input DMAs, critical-path first
                # DMA priority: weights needed by the early
                # projections, then xs blocks 0-1 (the fself preamble
                # matmuls run first and warm the PE HAM clock gate with
                # real work), then yb for the fy chain, then the rest.
                nc.sync.dma_start(t_wpack[:, 0:709], wpack[:, 0:709])
                nc.sync.dma_start(t_bias[:], bias32[:])
                nc.sync.dma_start(h1s_aug[64:65, :], ones[:, 0:4096])
                for g in range(2):
                    nc.sync.dma_start(t_xs[:, g * 1024:(g + 1) * 1024],
                                      xs[:, g * 1024:(g + 1) * 1024])
                t_yb = sbP.tile([128, 4096], F16)
                for c in range(4):
                    nc.sync.dma_start(t_yb[:, c * 512:(c + 1) * 512],
                                      yb[:, c * 512:(c + 1) * 512])
                nc.sync.dma_start(t_yb[:, 2048:3072], yb[:, 2048:3072])
                nc.sync.dma_start(t_yb[:, 3072:4096], yb[:, 3072:4096])
                for c in range(4):
                    nc.sync.dma_start(t_xl[:, c * 1024:(c + 1) * 1024],
                                      xl[:, c * 1024:(c + 1) * 1024])
                nc.sync.dma_start(t_wpack[:, 709:1093], wpack[:, 709:1093])
                for g in range(2, 8):
                    nc.sync.dma_start(t_xs[:, g * 1024:(g + 1) * 1024],
                                      xs[:, g * 1024:(g + 1) * 1024])

                # NOTE: no PE warm-up matmuls.  The PE is duty-cycle
                # limited (~70% sustained util; the firmware inserts
                # ~20.5us half-rate windows under load), so dummy
                # matmuls burn real budget and slow the whole kernel.

                # fself preamble first: these matmuls depend only on
                # the first DMAs and warm the PE HAM clock gate with
                # real work, so the projections run at full clock.
                for idx in range(2):
                    p = psP1.tile([64, 512], F32, tag="blk", bufs=4,
                                  name=f"pp_h1s_{idx}")
                    for a in range(2):
                        nc.tensor.matmul(p[:], t_ws1t[:, a * 64:(a + 1) * 64],
                                         t_xs[:, idx * 1024 + a * 512:
                                              idx * 1024 + a * 512 + 512],
                                         start=(a == 0), stop=(a == 1))
                    nc.vector.tensor_copy(
                        h1s_aug[0:64, idx * 512:idx * 512 + 512], p[:])
                for j in range(2):
                    p2 = psP1.tile([128, 66], F32, tag="blk", bufs=4,
                                   name=f"pp_fs_{j}")
                    nc.tensor.matmul(p2[:], h1s_aug[:, j * 128:(j + 1) * 128],
                                     t_ws2a, start=True, stop=True)
                    nc.vector.tensor_copy(fselfT[:, j * 65:(j + 1) * 65],
                                          p2[:, 0:65])

                # warm the ACT exp table early
                t_dum = sbP.tile([1, 32], F32)
                nc.vector.memset(t_dum[:], 0.0)
                t_dum2 = sbP.tile([1, 32], F32)
                nc.scalar.activation(t_dum2[:], t_dum[:], EXP)

                # ---- fy path: h1y = Wy1 @ y ; g = Wy2 @ h1y + by2 (raw)
                # and t2 = 0.25*g; banded upsample via fused
                # scalar_tensor_tensor (out = 0.75*a + 0.25-scaled b) ----
                h1y_s = sbP.tile([64, 1024], F16)
                fyc75 = sbP.tile([64, 1024], F32)
                fyc25 = sbP.tile([64, 1024], F32)
                for blk in range(2):
                    p = psP1.tile([64, 512], F32, tag="blk", bufs=4,
                                  name=f"p_h1y_{blk}")
                    for a in range(4):
                        nc.tensor.matmul(
                            p[:], t_wy1t[:, a * 64:(a + 1) * 64],
                            t_yb[:, blk * 2048 + a * 512:blk * 2048 + a * 512 + 512],
                            start=(a == 0), stop=(a == 3))
                    nc.scalar.activation(h1y_s[:, blk * 512:blk * 512 + 512],
                                         p[:], COPY)
                for blk in range(2):
                    p = psP1.tile([64, 512], F32, tag="blk", bufs=4,
                                  name=f"p_fyc_{blk}")
                    nc.tensor.matmul(p[:], t_wy2t,
                                     h1y_s[:, blk * 512:blk * 512 + 512],
                                     start=True, stop=True)
                    bs = slice(blk * 512, blk * 512 + 512)
                    nc.scalar.activation(fyc75[:, bs], p[:], IDENT,
                                         bias=t_by2a[:, 0:1], scale=0.75)
                    nc.scalar.activation(fyc25[:, bs], p[:], IDENT,
                                         bias=t_by2a[:, 1:2], scale=0.25)

                # H pass, 2 bands: [64, (32,32)] -> [64, (64,32)]
                fyH = sbM.tile([64, 2048], F32)
                t1v = fyc75[:].rearrange("p (h w) -> p h w", h=32)
                t2v = fyc25[:].rearrange("p (h w) -> p h w", h=32)
                fe = fyH[:].rearrange("p (h two w) -> p h two w", h=32, two=2)
                GADD = nc.gpsimd.tensor_add
                GADD(fe[:, 0, 0, :], t1v[:, 0, :], t2v[:, 0, :])
                GADD(fe[:, 1:16, 0, :], t1v[:, 1:16, :], t2v[:, 0:15, :])
                GADD(fe[:, 0:15, 1, :], t1v[:, 0:15, :], t2v[:, 1:16, :])
                GADD(fe[:, 16:32, 0, :], t1v[:, 16:32, :], t2v[:, 15:31, :])
                GADD(fe[:, 15:31, 1, :], t1v[:, 15:31, :], t2v[:, 16:32, :])
                GADD(fe[:, 31, 1, :], t1v[:, 31, :], t2v[:, 31, :])

                # 0.25-scaled fyH copy, 2 bands: rows 0..30 / 31..63
                u2 = sbM.tile([64, 2048], F32)
                u2v = u2[:].rearrange("p (h w) -> p h w", h=64)
                fyHv = fyH[:].rearrange("p (h w) -> p h w", h=64)
                nc.scalar.activation(u2[:, 0:31 * 32], fyH[:, 0:31 * 32],
                                     COPY, scale=0.25)
                nc.scalar.activation(u2[:, 31 * 32:2048], fyH[:, 31 * 32:2048],
                                     COPY, scale=0.25)

                # W pass + row-group duplication in 4 h-bands.  Band 0
                # (key chunks 0-7) is emitted at the end of phase 1; the
                # rest stream through half-0's hook so the first sims
                # are not blocked behind the whole fy chain on DVE.
                fw = fy2[0:64, :].rearrange("p (h w two) -> p h w two",
                                            h=64, two=2)
                _BANDS = ((slice(0, 16), 0, 1024),
                          (slice(16, 31), 1024, 1984),
                          (slice(31, 48), 1984, 3072),
                          (slice(48, 64), 3072, 4096))

                STT = nc.vector.scalar_tensor_tensor

                def wpass_band(b):
                    # STT stays on DVE: fy2 is f32r and its writers must
                    # produce rounded f32r (BIR verifier), which GpSimd
                    # only does via a slow software path.  The row-group
                    # duplication goes to ACT (also f32r-capable).
                    hs, c0, c1 = _BANDS[b]
                    nc.vector.tensor_copy(fw[:, hs, 0, 0], fyHv[:, hs, 0])
                    STT(fw[:, hs, 1:32, 0], fyHv[:, hs, 1:32], 0.75,
                        u2v[:, hs, 0:31], MUL, ADD)
                    STT(fw[:, hs, 0:31, 1], fyHv[:, hs, 0:31], 0.75,
                        u2v[:, hs, 1:32], MUL, ADD)
                    nc.vector.tensor_copy(fw[:, hs, 31, 1], fyHv[:, hs, 31])
                    nc.scalar.activation(fy2[64:128, c0:c1],
                                         fy2[0:64, c0:c1], COPY)

                # band 0 immediately -- before the fx path -- so the
                # first sims' ACT/DVE op-count waits don't extend past
                # the fx chain (which depends on later DMAs).
                wpass_band(0)

                # ---- fx path: h1x = Wx1 @ xl ; fx = Wx2 @ h1x + bx2 ----
                # Emitted BEFORE the W pass: the tile framework lowers
                # cross-engine deps as monotonic op-count waits, so the
                # first sims wait for the latest DVE op they depend on.
                # fx2 (incl. duplication) must precede the W-pass bands
                # in the DVE queue or sims stall on the whole fy chain.
                h1x_s = sbP.tile([64, 2048], F16)
                for blk in range(4):
                    p = psP1.tile([64, 512], F32, tag="blk", bufs=4,
                                  name=f"p_h1x_{blk}")
                    for a in range(2):
                        nc.tensor.matmul(
                            p[:], t_wx1t[:, a * 64:(a + 1) * 64],
                            t_xl[:, blk * 1024 + a * 512:blk * 1024 + a * 512 + 512],
                            start=(a == 0), stop=(a == 1))
                    nc.scalar.activation(h1x_s[:, blk * 512:blk * 512 + 512],
                                         p[:], COPY)
                for blk in range(4):
                    p = psP1.tile([64, 512], F32, tag="blk", bufs=4,
                                  name=f"p_fx_{blk}")
                    nc.tensor.matmul(p[:], t_wx2t,
                                     h1x_s[:, blk * 512:blk * 512 + 512],
                                     start=True, stop=True)
                    nc.vector.tensor_scalar_add(fx2[0:64, blk * 512:blk * 512 + 512],
                                                p[:], t_bx2)
                nc.scalar.activation(fx2[64:128, 0:1024],
                                      fx2[0:64, 0:1024], COPY)
                nc.scalar.activation(fx2[64:128, 1024:2048],
                                      fx2[0:64, 1024:2048], COPY)

                # more of the fself stream while the fy chain finishes
                # on ACT/DVE/GpSimd (PE is only ~60% busy here); blocks
                # 6-7 stay in the half-0 hook since their xs DMAs land
                # last.
                for idx in range(2, 6):
                    p = psP1.tile([64, 512], F32, tag="blk", bufs=4,
                                  name=f"pp_h1s_{idx}")
                    for a in range(2):
                        nc.tensor.matmul(p[:], t_ws1t[:, a * 64:(a + 1) * 64],
                                         t_xs[:, idx * 1024 + a * 512:
                                              idx * 1024 + a * 512 + 512],
                                         start=(a == 0), stop=(a == 1))
                    nc.vector.tensor_copy(
                        h1s_aug[0:64, idx * 512:idx * 512 + 512], p[:])
                    for j in (2 * idx - 2, 2 * idx - 1):
                        p2 = psP1.tile([128, 66], F32, tag="blk", bufs=4,
                                       name=f"pp_fs_{j}")
                        nc.tensor.matmul(p2[:],
                                         h1s_aug[:, j * 128:(j + 1) * 128],
                                         t_ws2a, start=True, stop=True)
                        nc.vector.tensor_copy(fselfT[:, j * 65:(j + 1) * 65],
                                              p2[:, 0:65])

            # ================= phase 2: attention =================
            fout_accs = {}

            def sim_unit(pool, j, h):
                ps = pool.tile([128, 1024], F32, tag="sim", bufs=2,
                               name=f"sim_{j}_{h}")
                nc.tensor.matmul(
                    ps[:, 0:512], fy2[0:64, j * 128:(j + 1) * 128],
                    fx2[0:64, h * 1024:h * 1024 + 512],
                    start=True, stop=True)
                nc.tensor.matmul(
                    ps[:, 512:1024], fy2[64:128, j * 128:(j + 1) * 128],
                    fx2[64:128, h * 1024 + 512:h * 1024 + 1024],
                    start=True, stop=True)
                return ps

            def exp_unit(st, j, h, on_dve):
                et = sbM.tile([128, 1024], BF16, tag="et", bufs=4,
                              name=f"et_{j}_{h}")
                if on_dve:
                    nc.vector.tensor_scalar(et[:].bitcast(I16), st[:],
                                            A16, B16, MUL, ADD)
                else:
                    nc.scalar.activation(et[:], st[:], EXP)
                return et

            def pv_unit(fout_acc, et, j):
                w = fselfT[:, j * 65:(j + 1) * 65]
                nc.tensor.matmul(fout_acc[:, 0:512], w, et[:, 0:512],
                                 start=(j == 0), stop=(j == KC - 1))
                nc.tensor.matmul(fout_acc[:, 512:1024], w, et[:, 512:1024],
                                 start=(j == 0), stop=(j == KC - 1))

            def half_loop(h, psB, hook, dve_set):
                fout_acc = fout_accs[h]
                sims = {}
                sims[0] = sim_unit(psB, 0, h)
                sims[1] = sim_unit(psB, 1, h)
                sims[2] = sim_unit(psB, 2, h)
                for j in range(KC):
                    if hook is not None:
                        hook(j)
                    et = exp_unit(sims.pop(j), j, h, j in dve_set)
                    pv_unit(fout_acc, et, j)
                    if j + 3 < KC:
                        sims[j + 3] = sim_unit(psB, j + 3, h)

            def pre_tail(h):
                # 1/Z scaling of fout into `scaled`; recip+broadcast for
                # both column groups first so the gpsimd broadcasts
                # overlap, then the two DVE muls.
                fout_acc = fout_accs[h]
                invzbs = []
                for s in range(2):
                    cs = slice(s * 512, (s + 1) * 512)
                    invz = sbM.tile([1, 512], F32, tag="zrow", bufs=2,
                                    name=f"invz_{h}_{s}")
                    nc.vector.reciprocal_approx_fast(invz[:], fout_acc[0:1, cs])
                    invzb = sbM.tile([128, 512], F32, tag="izb", bufs=2,
                                     name=f"invzb_{h}_{s}")
                    nc.gpsimd.partition_broadcast(invzb[:], invz[:])
                    invzbs.append(invzb)
                for s in range(2):
                    cs = slice(s * 512, (s + 1) * 512)
                    nc.vector.tensor_mul(
                        scaled[:, h * 1024 + s * 512:h * 1024 + (s + 1) * 512],
                        fout_acc[:, cs], invzbs[s][0:65, :])

            def up_quarter(psC, q):
                # up-projection + residual + output DMA for 512 query px
                for a in range(2):
                    p = psC.tile([128, 512], F32, tag="up", bufs=2,
                                 name=f"p_up_{q}_{a}")
                    nc.tensor.matmul(p[:], t_wupt[:, a * 128:(a + 1) * 128],
                                     scaled[:, q * 512:(q + 1) * 512],
                                     start=True, stop=True)
                    out_s = sbM.tile([128, 512], F32, tag="tail", bufs=4,
                                     name=f"out_s_{q}_{a}")
                    xv = t_xl[:, q * 1024 + a * 512:
                              q * 1024 + a * 512 + 512]
                    nc.vector.tensor_add(out_s[:], p[:], xv)
                    nc.sync.dma_start(
                        out[:, a * 2048 + q * 512:a * 2048 + (q + 1) * 512],
                        out_s[:])

            with tc.tile_pool(name="psA0", bufs=1, space="PSUM") as psA0:
                fout_accs[0] = psA0.tile([65, 1024], F32, name="fout0")
                with tc.tile_pool(name="psFS", bufs=1, space="PSUM") as psFS:

                    def fs_mms(j):
                        p = psFS.tile([128, 66], F32, tag="fs", bufs=2,
                                      name=f"p_fs_{j}")
                        nc.tensor.matmul(p[:],
                                         h1s_aug[:, j * 128:(j + 1) * 128],
                                         t_ws2a, start=True, stop=True)
                        nc.vector.tensor_copy(fselfT[:, j * 65:(j + 1) * 65],
                                              p[:, 0:65])

                    def h1s_mms(idx):
                        p = psFS.tile([64, 512], F32, tag="fs", bufs=2,
                                      name=f"p_h1s_{idx}")
                        for a in range(2):
                            nc.tensor.matmul(p[:],
                                             t_ws1t[:, a * 64:(a + 1) * 64],
                                             t_xs[:, idx * 1024 + a * 512:
                                                  idx * 1024 + a * 512 + 512],
                                             start=(a == 0), stop=(a == 1))
                        nc.vector.tensor_copy(
                            h1s_aug[0:64, idx * 512:idx * 512 + 512], p[:])

                    def hook0(j):
                        if j == 0:
                            wpass_band(1)
                        elif j == 4:
                            wpass_band(2)
                        elif j == 10:
                            wpass_band(3)
                        if j < 2:
                            h1s_mms(6 + j)
                        if 10 + 2 * j < KC:
                            fs_mms(10 + 2 * j)
                            fs_mms(11 + 2 * j)

                    with tc.tile_pool(name="psB0", bufs=1,
                                      space="PSUM") as psB0:
                        half_loop(0, psB0, hook0, DVE_H0)

                pre_tail(0)

            with tc.tile_pool(name="psA1", bufs=1, space="PSUM") as psA1:
                fout_accs[1] = psA1.tile([65, 1024], F32, name="fout1")
                with tc.tile_pool(name="psC", bufs=1, space="PSUM") as psC:

                    def hook1(j):
                        # half-0 tail interleaved into half-1's loop
                        if j == 6:
                            up_quarter(psC, 0)
                        elif j == 12:
                            up_quarter(psC, 1)

                    with tc.tile_pool(name="psB1", bufs=1,
                                      space="PSUM") as psB1:
                        half_loop(1, psB1, hook1, DVE_H1)
                    pre_tail(1)
                    up_quarter(psC, 2)
                    up_quarter(psC, 3)

    nc.compile()
    return nc


def _prep_maps(x, y, W_self1, b_self1, W_self2, b_self2, W_x1, b_x1, W_x2,
               b_x2, W_y1, b_y1, W_y2, b_y2, W_up, b_up):
    f64 = np.float64

    def fold(W2, b1, b2):
        return (W2.astype(f64) @ b1.astype(f64) + b2.astype(f64)).astype(np.float32)

    ws2a = np.zeros((65, 66), np.float16)
    ws2a[64, 0] = 1.0
    ws2a[0:64, 1:65] = W_self2.T.astype(np.float16)
    ws2a[64, 1:65] = fold(W_self2, b_self1, b_self2).astype(np.float16)
    bx2 = fold(W_x2, b_x1, b_x2).reshape(64, 1)
    _by2 = fold(W_y2, b_y1, b_y2).astype(np.float64)
    bias32 = np.ascontiguousarray(
        np.concatenate([bx2, 0.75 * _by2.reshape(64, 1),
                        0.25 * _by2.reshape(64, 1)], axis=1).astype(np.float32))

    ws1t = np.ascontiguousarray(
        W_self1.T.reshape(2, 128, 64).transpose(1, 0, 2).reshape(128, 128))
    wx1t = np.ascontiguousarray(
        W_x1.T.reshape(2, 128, 64).transpose(1, 0, 2).reshape(128, 128))
    wy1t = np.ascontiguousarray(
        W_y1.T.reshape(4, 128, 64).transpose(1, 0, 2).reshape(128, 256))
    wx2t = np.ascontiguousarray(W_x2.T)
    wy2t = np.ascontiguousarray(W_y2.T)
    wupt = np.ascontiguousarray(
        np.concatenate([b_up.reshape(1, 256), W_up.T], axis=0))
    wp = np.zeros((128, 1093), np.float16)
    wp[:, 0:128] = ws1t.astype(np.float16)
    wp[0:65, 128:194] = ws2a
    wp[:, 194:322] = wx1t.astype(np.float16)
    wp[0:64, 322:386] = wx2t.astype(np.float16)
    wp[:, 387:643] = wy1t.astype(np.float16)
    wp[0:64, 643:707] = wy2t.astype(np.float16)
    wp[0:65, 709:965] = wupt.astype(np.float16)

    _ONES = np.ones((1, 4096), np.float16)
    maps = []
    for b in range(B):
        xf = x[b].reshape(CX, N).astype(np.float16)             # [256, 4096]
        xs_h = np.ascontiguousarray(
            xf.reshape(2, 128, 8, 512).transpose(1, 2, 0, 3).reshape(128, 8192))
        yf = y[b].reshape(CY, NYC).astype(np.float16)
        yb_h = np.ascontiguousarray(
            yf.reshape(4, 128, 2, 512).transpose(1, 2, 0, 3).reshape(128, 4096))
        for half in range(2):
            xh = xf[:, half * NH:(half + 1) * NH]               # [256, 2048]
            xl_h = np.ascontiguousarray(
                xh.reshape(2, 128, 4, 512).transpose(1, 2, 0, 3).reshape(128, 4096))
            maps.append({
                "xs": xs_h, "xl": xl_h, "yb": yb_h,
                "wpack": wp, "bias32": bias32, "ones": _ONES,
            })
    return maps


def _run(inputs, trace=False, trace_kwargs=None):
    if "nc" not in _CACHE:
        _CACHE["nc"] = _build()
    nc = _CACHE["nc"]
    maps = _prep_maps(**inputs)
    res = run_bass_kernel_spmd(nc, maps, list(range(8)), trace=trace,
                               **(trace_kwargs or {}))
    outs = np.empty((B, CX, H, W), np.float32)
    for b in range(B):
        for half in range(2):
            o = res.results[2 * b + half]["out"]                # [128, 4096]
            oh = o.reshape(128, 2, NH).transpose(1, 0, 2).reshape(CX, NH)
            outs[b, :, :, :].reshape(CX, N)[:, half * NH:(half + 1) * NH] = oh
    return outs, res


def kernel(**inputs):
    outs, _ = _run(inputs, trace=False)
    return outs


# revision 29
# speedup vs baseline: 1.2018x; 1.0170x over previous
"""Trainium2 Bass kernel for nn_BCA_17274358465235.

Module: out = x + conv1x1_up( softmax(fx @ fy_up^T) @ fself ) with
fx/fself = 2-layer 1x1-conv projections of x, fy = projection of
bilinearly-upsampled y.  B=4, CX=256, CY=512, CM=64, H=W=64 (N=4096
tokens), HY=WY=32.

Sharding: 8 cores = batch(4) x query-row-half(2).  Each core holds all
4096 keys (fy/fself replicated per batch) and 2048 query rows.  No
collectives.  One program for all cores (SPMD).

Per-core algorithm (layouts chosen so no transposes are needed):
  fself^T[key, c]  via second projection layer emitted transposed
  sim^T[key, row] = fy_f[:, keys]^T @ fx[:, rows]   (fp32r matmuls,
      two key-chunks packed into PE row-groups 0-1 / 2-3)
  exp: split between ACT (exact, bf16 out) and DVE (Schraudolph:
      int16(A*sim+B) bit-cast as bf16; end-to-end rel err stays ~1.2e-2)
  fout^T[c, row] += fself^T_chunk^T @ exp_chunk   (PSUM accumulation,
      ones-column in fself^T produces the softmax denominator Z free)
  out = x + W_up @ (fout^T * (1/Z)) + b_up   (b_up via ones-row in W_up)

Performance changes over the first working kernel:
  - exp split ACT/DVE: the ACT engine alone paced the main loop at
    ~1.22us/tile; DVE takes a share via the Schraudolph step.
  - half-0's 1/Z scaling + up-projection + residual + output DMA are
    hooked into half-1's attention loop; only half 1's tail is serial.
  - fy upsample fused with scalar_tensor_tensor (drops the 0.75-scaled
    copy and half the fyc work); W-pass bands 1-3 + the fx path are
    ordered so the first sims don't wait on the whole fy chain (deps
    lower to monotonic per-engine op-count waits).
  - no PE warm-up/keepalive matmuls: the PE is power duty-cycle capped
    (~70%; ~20.5us half-rate windows under sustained load), so dummy
    matmuls cost real budget.
"""
import sys

for _p in ("/opt/pypackages", "/opt/trn_rl_repo"):
    if _p not in sys.path:
        sys.path.insert(0, _p)

import numpy as np

import concourse.bacc as bacc
import concourse.mybir as mybir
import concourse.tile as tile
from concourse.bass_utils import run_bass_kernel_spmd

F32 = mybir.dt.float32
F32R = mybir.dt.float32r
F16 = mybir.dt.float16
BF16 = mybir.dt.bfloat16
I16 = mybir.dt.int16
EXP = mybir.ActivationFunctionType.Exp
COPY = mybir.ActivationFunctionType.Copy
IDENT = mybir.ActivationFunctionType.Identity
MUL = mybir.AluOpType.mult
ADD = mybir.AluOpType.add

B, CX, CY, CM = 4, 256, 512, 64
H = W = 64
HY = WY = 32
N = H * W              # 4096 tokens
NH = N // 2            # 2048 query rows per core
NYC = HY * WY          # 1024 coarse tokens
KC = N // 128          # 32 key chunks

# Schraudolph exp in bf16 bit-domain: bf16_bits(e^x) ~ int16(A16*x + B16)
# (the DVE f32->int16 output conversion rounds to nearest).
A16 = float((1 << 7) / np.log(2.0))
B16 = 16250.12

# Which iteration indices (0..31) of each half-loop run exp on DVE
# (Schraudolph); the rest run exact exp on ACT.  DVE also carries the
# fselfT copies + deferred fy W-pass bands (half 0) and the pre-tail /
# residual work (half 1), so its share starts late in each half.
DVE_H0 = frozenset(j for j in range(KC) if j % 2 == 1 and 5 <= j < 29)
DVE_H1 = frozenset(j for j in range(KC) if j % 2 == 1 and 5 <= j < 28)

_CACHE = {}


def _build():
    nc = bacc.Bacc("TRN2", target_bir_lowering=False, debug=False,
                   enable_asserts=False)

    # ---- DRAM I/O (per-core layouts pre-arranged on host) ----
    # xs: [128, 8 * 1024] block-major: block g = [ch0-127 | ch128-255] of
    #     pixel columns g*512..(g+1)*512  (for fself over the full image)
    xs = nc.dram_tensor("xs", [128, 8192], F16, kind="ExternalInput").ap()
    # xl: [128, 2 * 2048] ch-chunk-major: this core's 2048 query pixels
    xl = nc.dram_tensor("xl", [128, 4096], F16, kind="ExternalInput").ap()
    # yb: [128, 4 * 1024] ch-chunk-major
    yb = nc.dram_tensor("yb", [128, 4096], F16, kind="ExternalInput").ap()
    wpack = nc.dram_tensor("wpack", [128, 1093], F16, kind="ExternalInput").ap()
    bias32 = nc.dram_tensor("bias32", [64, 3], F32, kind="ExternalInput").ap()
    ones = nc.dram_tensor("ones", [1, 4096], F16, kind="ExternalInput").ap()
    # out: [128, 2 * 2048] ch-chunk-major
    out = nc.dram_tensor("out", [128, 4096], F32, kind="ExternalOutput").ap()

    with tile.TileContext(nc) as tc:
        with tc.tile_pool(name="sbW", bufs=1) as sbW, \
             tc.tile_pool(name="sbM", bufs=1) as sbM:
            # ---- long-lived SBUF ----
            t_xs = sbM.tile([128, 8192], F16)      # full x for fself stream
            t_xl = sbM.tile([128, 4096], F16)      # fx input + residual
            fy2 = sbM.tile([128, 4096], F16)       # upsampled fy, duplicated
            fx2 = sbM.tile([128, 2048], F16)       # fx, duplicated
            fselfT = sbM.tile([128, 65 * KC], BF16)
            h1s_aug = sbM.tile([65, 4096], F16)    # W_self1 @ x with ones row
            scaled = sbM.tile([65, 2048], F16)     # [Z/Z; fout/Z] per row
            t_bias = sbM.tile([64, 3], F32)        # bx2 | 0.75*by2 | 0.25*by2

            # ---- weights (single packed blob) ----
            t_wpack = sbW.tile([128, 1093], F16)
            t_ws1t = t_wpack[:, 0:128]
            t_ws2a = t_wpack[0:65, 128:194]
            t_wx1t = t_wpack[:, 194:322]
            t_wx2t = t_wpack[0:64, 322:386]
            t_wy1t = t_wpack[:, 387:643]
            t_wy2t = t_wpack[0:64, 643:707]
            t_wupt = t_wpack[0:65, 709:965]
            t_bx2 = t_bias[:, 0:1]
            t_by2a = t_bias[:, 1:3]

            # ================= phase 1: projections =================
            with tc.tile_pool(name="sbP", bufs=1) as sbP, \
                 tc.tile_pool(name="psP1", bufs=1, space="PSUM") as psP1:
                # input DMAs, critical-path first
                # DMA priority: weights needed by the early
                # projections, then xs blocks 0-1 (the fself preamble
                # matmuls run first and warm the PE HAM clock gate with
                # real work), then yb for the fy chain, then the rest.
                nc.sync.dma_start(t_wpack[:, 0:709], wpack[:, 0:709])
                nc.sync.dma_start(t_bias[:], bias32[:])
                nc.sync.dma_start(h1s_aug[64:65, :], ones[:, 0:4096])
                for g in range(2):
                    nc.scalar.dma_start(t_xs[:, g * 1024:(g + 1) * 1024],
                                        xs[:, g * 1024:(g + 1) * 1024])
                t_yb = sbP.tile([128, 4096], F16)
                for c in range(4):
                    nc.sync.dma_start(t_yb[:, c * 512:(c + 1) * 512],
                                      yb[:, c * 512:(c + 1) * 512])
                nc.sync.dma_start(t_yb[:, 2048:3072], yb[:, 2048:3072])
                nc.sync.dma_start(t_yb[:, 3072:4096], yb[:, 3072:4096])
                for c in range(4):
                    nc.sync.dma_start(t_xl[:, c * 1024:(c + 1) * 1024],
                                      xl[:, c * 1024:(c + 1) * 1024])
                nc.sync.dma_start(t_wpack[:, 709:1093], wpack[:, 709:1093])
                for g in range(2, 8):
                    nc.sync.dma_start(t_xs[:, g * 1024:(g + 1) * 1024],
                                      xs[:, g * 1024:(g + 1) * 1024])

                # NOTE: no PE warm-up matmuls.  The PE is duty-cycle
                # limited (~70% sustained util; the firmware inserts
                # ~20.5us half-rate windows under load), so dummy
                # matmuls burn real budget and slow the whole kernel.

                # fself preamble first: these matmuls depend only on
                # the first DMAs and warm the PE HAM clock gate with
                # real work, so the projections run at full clock.
                for idx in range(2):
                    p = psP1.tile([64, 512], F32, tag="blk", bufs=4,
                                  name=f"pp_h1s_{idx}")
                    for a in range(2):
                        nc.tensor.matmul(p[:], t_ws1t[:, a * 64:(a + 1) * 64],
                                         t_xs[:, idx * 1024 + a * 512:
                                              idx * 1024 + a * 512 + 512],
                                         start=(a == 0), stop=(a == 1))
                    nc.vector.tensor_copy(
                        h1s_aug[0:64, idx * 512:idx * 512 + 512], p[:])
                for j in range(2):
                    p2 = psP1.tile([128, 66], F32, tag="blk", bufs=4,
                                   name=f"pp_fs_{j}")
                    nc.tensor.matmul(p2[:], h1s_aug[:, j * 128:(j + 1) * 128],
                                     t_ws2a, start=True, stop=True)
                    nc.vector.tensor_copy(fselfT[:, j * 65:(j + 1) * 65],
                                          p2[:, 0:65])

                # warm the ACT exp table early
                t_dum = sbP.tile([1, 32], F32)
                nc.vector.memset(t_dum[:], 0.0)
                t_dum2 = sbP.tile([1, 32], F32)
                nc.scalar.activation(t_dum2[:], t_dum[:], EXP)

                # ---- fy path: h1y = Wy1 @ y ; g = Wy2 @ h1y + by2 (raw)
                # and t2 = 0.25*g; banded upsample via fused
                # scalar_tensor_tensor (out = 0.75*a + 0.25-scaled b) ----
                h1y_s = sbP.tile([64, 1024], F16)
                fyc75 = sbP.tile([64, 1024], F32)
                fyc25 = sbP.tile([64, 1024], F32)
                for blk in range(2):
                    p = psP1.tile([64, 512], F32, tag="blk", bufs=4,
                                  name=f"p_h1y_{blk}")
                    for a in range(4):
                        nc.tensor.matmul(
                            p[:], t_wy1t[:, a * 64:(a + 1) * 64],
                            t_yb[:, blk * 2048 + a * 512:blk * 2048 + a * 512 + 512],
                            start=(a == 0), stop=(a == 3))
                    nc.scalar.activation(h1y_s[:, blk * 512:blk * 512 + 512],
                                         p[:], COPY)
                for blk in range(2):
                    p = psP1.tile([64, 512], F32, tag="blk", bufs=4,
                                  name=f"p_fyc_{blk}")
                    nc.tensor.matmul(p[:], t_wy2t,
                                     h1y_s[:, blk * 512:blk * 512 + 512],
                                     start=True, stop=True)
                    bs = slice(blk * 512, blk * 512 + 512)
                    nc.scalar.activation(fyc75[:, bs], p[:], IDENT,
                                         bias=t_by2a[:, 0:1], scale=0.75)
                    nc.scalar.activation(fyc25[:, bs], p[:], IDENT,
                                         bias=t_by2a[:, 1:2], scale=0.25)

                # H pass, 2 bands: [64, (32,32)] -> [64, (64,32)]
                fyH = sbM.tile([64, 2048], F32)
                t1v = fyc75[:].rearrange("p (h w) -> p h w", h=32)
                t2v = fyc25[:].rearrange("p (h w) -> p h w", h=32)
                fe = fyH[:].rearrange("p (h two w) -> p h two w", h=32, two=2)
                GADD = nc.gpsimd.tensor_add
                GADD(fe[:, 0, 0, :], t1v[:, 0, :], t2v[:, 0, :])
                GADD(fe[:, 1:16, 0, :], t1v[:, 1:16, :], t2v[:, 0:15, :])
                GADD(fe[:, 0:15, 1, :], t1v[:, 0:15, :], t2v[:, 1:16, :])
                GADD(fe[:, 16:32, 0, :], t1v[:, 16:32, :], t2v[:, 15:31, :])
                GADD(fe[:, 15:31, 1, :], t1v[:, 15:31, :], t2v[:, 16:32, :])
                GADD(fe[:, 31, 1, :], t1v[:, 31, :], t2v[:, 31, :])

                # 0.25-scaled fyH copy, 2 bands: rows 0..30 / 31..63
                u2 = sbM.tile([64, 2048], F32)
                u2v = u2[:].rearrange("p (h w) -> p h w", h=64)
                fyHv = fyH[:].rearrange("p (h w) -> p h w", h=64)
                nc.scalar.activation(u2[:, 0:31 * 32], fyH[:, 0:31 * 32],
                                     COPY, scale=0.25)
                nc.scalar.activation(u2[:, 31 * 32:2048], fyH[:, 31 * 32:2048],
                                     COPY, scale=0.25)

                # W pass + row-group duplication in 4 h-bands.  Band 0
                # (key chunks 0-7) is emitted at the end of phase 1; the
                # rest stream through half-0's hook so the first sims
                # are not blocked behind the whole fy chain on DVE.
                fw = fy2[0:64, :].rearrange("p (h w two) -> p h w two",
                                            h=64, two=2)
                _BANDS = ((slice(0, 16), 0, 1024),
                          (slice(16, 31), 1024, 1984),
                          (slice(31, 48), 1984, 3072),
                          (slice(48, 64), 3072, 4096))

                STT = nc.vector.scalar_tensor_tensor

                def wpass_band(b):
                    # STT stays on DVE: fy2 is f32r and its writers must
                    # produce rounded f32r (BIR verifier), which GpSimd
                    # only does via a slow software path.  The row-group
                    # duplication goes to ACT (also f32r-capable).
                    hs, c0, c1 = _BANDS[b]
                    nc.vector.tensor_copy(fw[:, hs, 0, 0], fyHv[:, hs, 0])
                    STT(fw[:, hs, 1:32, 0], fyHv[:, hs, 1:32], 0.75,
                        u2v[:, hs, 0:31], MUL, ADD)
                    STT(fw[:, hs, 0:31, 1], fyHv[:, hs, 0:31], 0.75,
                        u2v[:, hs, 1:32], MUL, ADD)
                    nc.vector.tensor_copy(fw[:, hs, 31, 1], fyHv[:, hs, 31])
                    nc.scalar.activation(fy2[64:128, c0:c1],
                                         fy2[0:64, c0:c1], COPY)

                # band 0 immediately -- before the fx path -- so the
                # first sims' ACT/DVE op-count waits don't extend past
                # the fx chain (which depends on later DMAs).
                wpass_band(0)

                # ---- fx path: h1x = Wx1 @ xl ; fx = Wx2 @ h1x + bx2 ----
                # Emitted BEFORE the W pass: the tile framework lowers
                # cross-engine deps as monotonic op-count waits, so the
                # first sims wait for the latest DVE op they depend on.
                # fx2 (incl. duplication) must precede the W-pass bands
                # in the DVE queue or sims stall on the whole fy chain.
                h1x_s = sbP.tile([64, 2048], F16)
                for blk in range(4):
                    p = psP1.tile([64, 512], F32, tag="blk", bufs=4,
                                  name=f"p_h1x_{blk}")
                    for a in range(2):
                        nc.tensor.matmul(
                            p[:], t_wx1t[:, a * 64:(a + 1) * 64],
                            t_xl[:, blk * 1024 + a * 512:blk * 1024 + a * 512 + 512],
                            start=(a == 0), stop=(a == 1))
                    nc.scalar.activation(h1x_s[:, blk * 512:blk * 512 + 512],
                                         p[:], COPY)
                for blk in range(4):
                    p = psP1.tile([64, 512], F32, tag="blk", bufs=4,
                                  name=f"p_fx_{blk}")
                    nc.tensor.matmul(p[:], t_wx2t,
                                     h1x_s[:, blk * 512:blk * 512 + 512],
                                     start=True, stop=True)
                    nc.vector.tensor_scalar_add(fx2[0:64, blk * 512:blk * 512 + 512],
                                                p[:], t_bx2)
                nc.scalar.activation(fx2[64:128, 0:1024],
                                      fx2[0:64, 0:1024], COPY)
                nc.scalar.activation(fx2[64:128, 1024:2048],
                                      fx2[0:64, 1024:2048], COPY)

                # more of the fself stream while the fy chain finishes
                # on ACT/DVE/GpSimd (PE is only ~60% busy here); blocks
                # 6-7 stay in the half-0 hook since their xs DMAs land
                # last.
                for idx in range(2, 6):
                    p = psP1.tile([64, 512], F32, tag="blk", bufs=4,
                                  name=f"pp_h1s_{idx}")
                    for a in range(2):
                        nc.tensor.matmul(p[:], t_ws1t[:, a * 64:(a + 1) * 64],
                                         t_xs[:, idx * 1024 + a * 512:
                                              idx * 1024 + a * 512 + 512],
                                         start=(a == 0), stop=(a == 1))
                    nc.vector.tensor_copy(
                        h1s_aug[0:64, idx * 512:idx * 512 + 512], p[:])
                    for j in (2 * idx - 2, 2 * idx - 1):
                        p2 = psP1.tile([128, 66], F32, tag="blk", bufs=4,
                                       name=f"pp_fs_{j}")
                        nc.tensor.matmul(p2[:],
                                         h1s_aug[:, j * 128:(j + 1) * 128],
                                         t_ws2a, start=True, stop=True)
                        nc.vector.tensor_copy(fselfT[:, j * 65:(j + 1) * 65],
                                              p2[:, 0:65])

            # ================= phase 2: attention =================
            fout_accs = {}

            def sim_unit(pool, j, h):
                ps = pool.tile([128, 1024], F32, tag="sim", bufs=2,
                               name=f"sim_{j}_{h}")
                nc.tensor.matmul(
                    ps[:, 0:512], fy2[0:64, j * 128:(j + 1) * 128],
                    fx2[0:64, h * 1024:h * 1024 + 512],
                    start=True, stop=True)
                nc.tensor.matmul(
                    ps[:, 512:1024], fy2[64:128, j * 128:(j + 1) * 128],
                    fx2[64:128, h * 1024 + 512:h * 1024 + 1024],
                    start=True, stop=True)
                return ps

            def exp_unit(st, j, h, on_dve):
                et = sbM.tile([128, 1024], BF16, tag="et", bufs=4,
                              name=f"et_{j}_{h}")
                if on_dve:
                    nc.vector.tensor_scalar(et[:].bitcast(I16), st[:],
                                            A16, B16, MUL, ADD)
                else:
                    nc.scalar.activation(et[:], st[:], EXP)
                return et

            def pv_unit(fout_acc, et, j):
                w = fselfT[:, j * 65:(j + 1) * 65]
                nc.tensor.matmul(fout_acc[:, 0:512], w, et[:, 0:512],
                                 start=(j == 0), stop=(j == KC - 1))
                nc.tensor.matmul(fout_acc[:, 512:1024], w, et[:, 512:1024],
                                 start=(j == 0), stop=(j == KC - 1))

            def half_loop(h, psB, hook, dve_set):
                fout_acc = fout_accs[h]
                sims = {}
                sims[0] = sim_unit(psB, 0, h)
                sims[1] = sim_unit(psB, 1, h)
                sims[2] = sim_unit(psB, 2, h)
                for j in range(KC):
                    if hook is not None:
                        hook(j)
                    et = exp_unit(sims.pop(j), j, h, j in dve_set)
                    pv_unit(fout_acc, et, j)
                    if j + 3 < KC:
                        sims[j + 3] = sim_unit(psB, j + 3, h)

            def pre_tail(h):
                # 1/Z scaling of fout into `scaled`; recip+broadcast for
                # both column groups first so the gpsimd broadcasts
                # overlap, then the two DVE muls.
                fout_acc = fout_accs[h]
                invzbs = []
                for s in range(2):
                    cs = slice(s * 512, (s + 1) * 512)
                    invz = sbM.tile([1, 512], F32, tag="zrow", bufs=2,
                                    name=f"invz_{h}_{s}")
                    nc.vector.reciprocal_approx_fast(invz[:], fout_acc[0:1, cs])
                    invzb = sbM.tile([128, 512], F32, tag="izb", bufs=2,
                                     name=f"invzb_{h}_{s}")
                    nc.gpsimd.partition_broadcast(invzb[:], invz[:])
                    invzbs.append(invzb)
                for s in range(2):
                    cs = slice(s * 512, (s + 1) * 512)
                    nc.vector.tensor_mul(
                        scaled[:, h * 1024 + s * 512:h * 1024 + (s + 1) * 512],
                        fout_acc[:, cs], invzbs[s][0:65, :])

            def up_quarter(psC, q):
                # up-projection + residual + output DMA for 512 query px
                for a in range(2):
                    p = psC.tile([128, 512], F32, tag="up", bufs=2,
                                 name=f"p_up_{q}_{a}")
                    nc.tensor.matmul(p[:], t_wupt[:, a * 128:(a + 1) * 128],
                                     scaled[:, q * 512:(q + 1) * 512],
                                     start=True, stop=True)
                    out_s = sbM.tile([128, 512], F32, tag="tail", bufs=4,
                                     name=f"out_s_{q}_{a}")
                    xv = t_xl[:, q * 1024 + a * 512:
                              q * 1024 + a * 512 + 512]
                    nc.vector.tensor_add(out_s[:], p[:], xv)
                    nc.sync.dma_start(
                        out[:, a * 2048 + q * 512:a * 2048 + (q + 1) * 512],
                        out_s[:])

            with tc.tile_pool(name="psA0", bufs=1, space="PSUM") as psA0:
                fout_accs[0] = psA0.tile([65, 1024], F32, name="fout0")
                with tc.tile_pool(name="psFS", bufs=1, space="PSUM") as psFS:

                    def fs_mms(j):
                        p = psFS.tile([128, 66], F32, tag="fs", bufs=2,
                                      name=f"p_fs_{j}")
                        nc.tensor.matmul(p[:],
                                         h1s_aug[:, j * 128:(j + 1) * 128],
                                         t_ws2a, start=True, stop=True)
                        nc.vector.tensor_copy(fselfT[:, j * 65:(j + 1) * 65],
                                              p[:, 0:65])

                    def h1s_mms(idx):
                        p = psFS.tile([64, 512], F32, tag="fs", bufs=2,
                                      name=f"p_h1s_{idx}")
                        for a in range(2):
                            nc.tensor.matmul(p[:],
                                             t_ws1t[:, a * 64:(a + 1) * 64],
                                             t_xs[:, idx * 1024 + a * 512:
                                                  idx * 1024 + a * 512 + 512],
                                             start=(a == 0), stop=(a == 1))
                        nc.vector.tensor_copy(
                            h1s_aug[0:64, idx * 512:idx * 512 + 512], p[:])

                    def hook0(j):
                        if j == 0:
                            wpass_band(1)
                        elif j == 4:
                            wpass_band(2)
                        elif j == 10:
                            wpass_band(3)
                        if j < 2:
                            h1s_mms(6 + j)
                        if 10 + 2 * j < KC:
                            fs_mms(10 + 2 * j)
                            fs_mms(11 + 2 * j)

                    with tc.tile_pool(name="psB0", bufs=1,
                                      space="PSUM") as psB0:
                        half_loop(0, psB0, hook0, DVE_H0)

                pre_tail(0)

            with tc.tile_pool(name="psA1", bufs=1, space="PSUM") as psA1:
                fout_accs[1] = psA1.tile([65, 1024], F32, name="fout1")
                with tc.tile_pool(name="psC", bufs=1, space="PSUM") as psC:

                    def hook1(j):
                        # half-0 tail interleaved into half-1's loop
                        if j == 6:
                            up_quarter(psC, 0)
                        elif j == 12:
                            up_quarter(psC, 1)

                    with tc.tile_pool(name="psB1", bufs=1,
                                      space="PSUM") as psB1:
                        half_loop(1, psB1, hook1, DVE_H1)
                    pre_tail(1)
                    up_quarter(psC, 2)
                    up_quarter(psC, 3)

    nc.compile()
    return nc


def _prep_maps(x, y, W_self1, b_self1, W_self2, b_self2, W_x1, b_x1, W_x2,
               b_x2, W_y1, b_y1, W_y2, b_y2, W_up, b_up):
    f64 = np.float64

    def fold(W2, b1, b2):
        return (W2.astype(f64) @ b1.astype(f64) + b2.astype(f64)).astype(np.float32)

    ws2a = np.zeros((65, 66), np.float16)
    ws2a[64, 0] = 1.0
    ws2a[0:64, 1:65] = W_self2.T.astype(np.float16)
    ws2a[64, 1:65] = fold(W_self2, b_self1, b_self2).astype(np.float16)
    bx2 = fold(W_x2, b_x1, b_x2).reshape(64, 1)
    _by2 = fold(W_y2, b_y1, b_y2).astype(np.float64)
    bias32 = np.ascontiguousarray(
        np.concatenate([bx2, 0.75 * _by2.reshape(64, 1),
                        0.25 * _by2.reshape(64, 1)], axis=1).astype(np.float32))

    ws1t = np.ascontiguousarray(
        W_self1.T.reshape(2, 128, 64).transpose(1, 0, 2).reshape(128, 128))
    wx1t = np.ascontiguousarray(
        W_x1.T.reshape(2, 128, 64).transpose(1, 0, 2).reshape(128, 128))
    wy1t = np.ascontiguousarray(
        W_y1.T.reshape(4, 128, 64).transpose(1, 0, 2).reshape(128, 256))
    wx2t = np.ascontiguousarray(W_x2.T)
    wy2t = np.ascontiguousarray(W_y2.T)
    wupt = np.ascontiguousarray(
        np.concatenate([b_up.reshape(1, 256), W_up.T], axis=0))
    wp = np.zeros((128, 1093), np.float16)
    wp[:, 0:128] = ws1t.astype(np.float16)
    wp[0:65, 128:194] = ws2a
    wp[:, 194:322] = wx1t.astype(np.float16)
    wp[0:64, 322:386] = wx2t.astype(np.float16)
    wp[:, 387:643] = wy1t.astype(np.float16)
    wp[0:64, 643:707] = wy2t.astype(np.float16)
    wp[0:65, 709:965] = wupt.astype(np.float16)

    _ONES = np.ones((1, 4096), np.float16)
    maps = []
    for b in range(B):
        xf = x[b].reshape(CX, N).astype(np.float16)             # [256, 4096]
        xs_h = np.ascontiguousarray(
            xf.reshape(2, 128, 8, 512).transpose(1, 2, 0, 3).reshape(128, 8192))
        yf = y[b].reshape(CY, NYC).astype(np.float16)
        yb_h = np.ascontiguousarray(
            yf.reshape(4, 128, 2, 512).transpose(1, 2, 0, 3).reshape(128, 4096))
        for half in range(2):
            xh = xf[:, half * NH:(half + 1) * NH]               # [256, 2048]
            xl_h = np.ascontiguousarray(
                xh.reshape(2, 128, 4, 512).transpose(1, 2, 0, 3).reshape(128, 4096))
            maps.append({
                "xs": xs_h, "xl": xl_h, "yb": yb_h,
                "wpack": wp, "bias32": bias32, "ones": _ONES,
            })
    return maps


def _run(inputs, trace=False, trace_kwargs=None):
    if "nc" not in _CACHE:
        _CACHE["nc"] = _build()
    nc = _CACHE["nc"]
    maps = _prep_maps(**inputs)
    res = run_bass_kernel_spmd(nc, maps, list(range(8)), trace=trace,
                               **(trace_kwargs or {}))
    outs = np.empty((B, CX, H, W), np.float32)
    for b in range(B):
        for half in range(2):
            o = res.results[2 * b + half]["out"]                # [128, 4096]
            oh = o.reshape(128, 2, NH).transpose(1, 0, 2).reshape(CX, NH)
            outs[b, :, :, :].reshape(CX, N)[:, half * NH:(half + 1) * NH] = oh
    return outs, res


def kernel(**inputs):
    outs, _ = _run(inputs, trace=False)
    return outs
